# revision 1
# baseline (speedup 1.0000x reference)
"""DigitCaps dynamic-routing kernel for Trainium2 (8 NeuronCores, Bass/Tile).

Math (per routing iteration, reformulated to avoid materializing u_hat):
    u_hat[b,i,j,u] = sum_k W[i,j,u,k] * x[b,k,i]
    s[b,ju]  = sum_{ki} X[ki,b] * (c[i,j] * W[ki,ju])          (PE matmul, K=9216)
    v        = squash(s)  with the reference's quirky j-axis norm
    G[ki,ju] = sum_b X[b,ki] * v[b,ju]                         (PE matmul, K=64)
    b[i,j]   = sum_{k,u} W[ki,ju] * G[ki,ju]                   (DVE STT w/ accum)
    b is AllReduced (sum) over the 8 cores each iteration (batch mean).

Sharding: data-parallel over batch B=512 -> 64 rows per core; W replicated.
Key perf choices (measured on HW):
  - fp32 s-chain: accumulation chains hide the fp32 LDWEIGHTS; bf16 with
    M=64 stationary defeats FWL and runs 3x slower.
  - bf16 G-matmuls: M=128 stationary enables fast weight load (68ns/MM vs
    533ns fp32); b-update averages 128*512 terms so bf16 noise is harmless.
  - G PSUM rounds are evacuated to SBUF by ACT in bank-halves so the next
    round's matmuls don't serialize behind DVE reads (PSUM bank hazard).
  - squash runs on DVE except Sqrt (ACT LUT-table reloads cost ~1.3us).
  - the per-iteration b AllReduce is split in two halves so the first
    collective overlaps the tail of the b-update compute.
"""

import sys

sys.path.insert(0, "/opt/trn_rl_repo")

from contextlib import ExitStack

import numpy as np

B = 512
NCORES = 8
BL = B // NCORES  # 64 local batch rows
K = 8             # in_units (primary capsule dim)
IC = 1152         # in_channels (number of primary capsules)
J = 10            # num_units (output capsules)
U = 16            # unit_size
JU = J * U        # 160
NT = IC // 128    # 9 i-chunks of 128
NKT = K * NT      # 72 ki-chunks of 128
BETA = 1.45
NUM_ROUTING = 3
NT_A = 5          # t2 chunks in the first AllReduce half
IC_A = NT_A * 128

_CACHE = {}


def _build_nc():
    import concourse.bass as bass
    import concourse.tile as tile
    from concourse import bacc, mybir
    from concourse.masks import make_identity

    f32 = mybir.dt.float32
    bf16 = mybir.dt.bfloat16
    Alu = mybir.AluOpType
    Act = mybir.ActivationFunctionType

    nc = bacc.Bacc("TRN2", target_bir_lowering=False, debug=False,
                   num_devices=NCORES)

    xs = nc.dram_tensor("xs", [BL, K, IC], f32, kind="ExternalInput").ap()
    w = nc.dram_tensor("w", [IC, J, U, K], f32, kind="ExternalInput").ap()
    out = nc.dram_tensor("out", [BL, J, 4, 4], f32, kind="ExternalOutput").ap()

    xs_flat = xs.rearrange("b k i -> b (k i)")          # [64, 9216]
    w_r = w.rearrange("(t p) j u k -> p t (j u k)", p=128)  # [128, 9, 1280]
    out_flat = out.rearrange("b j g h -> b (j g h)")    # [64, 160]

    with tile.TileContext(nc) as tc, ExitStack() as ctx:
        consts = ctx.enter_context(tc.tile_pool(name="consts", bufs=1))
        small = ctx.enter_context(tc.tile_pool(name="small", bufs=2))
        scratch = ctx.enter_context(tc.tile_pool(name="scratch", bufs=8))
        psum = ctx.enter_context(tc.tile_pool(name="psum", bufs=1, space="PSUM"))
        dram = ctx.enter_context(tc.tile_pool(name="dram", bufs=1, space="DRAM"))

        # ---- persistent SBUF tensors ----
        x2 = consts.tile([BL, K * IC], f32)          # x[b, (k i)]
        x2b = consts.tile([BL, K * IC], bf16)        # bf16 copy for G matmuls
        x1 = consts.tile([128, NKT, BL], f32)        # x^T per ki-chunk
        w_nat = consts.tile([128, NT, J * U * K], f32)  # W natural layout
        wp = consts.tile([128, NKT, JU], f32)        # c-scaled W (matmul rhs)
        crep = consts.tile([128, NT, JU], f32)       # c broadcast over u
        ident = consts.tile([BL, BL], f32)
        ones = consts.tile([128, 128], f32)

        # one PSUM tensor = all 8 banks; everything slices into it
        pall = psum.tile([128, K, 512], f32)

        # W as [p, t2, j, u, k] view for strided reads
        w5 = w_nat.rearrange("p t (j u k) -> p t j u k", j=J, u=U)

        # ---- loads: split into small pieces so transfers spread across the
        # 16 DMA engines (a single transfer lands on one engine @~22GB/s) ----
        HIC = IC // 2
        for k in range(K):
            for h in range(2):
                nc.sync.dma_start(
                    out=x2[:, k * IC + h * HIC:k * IC + (h + 1) * HIC],
                    in_=xs_flat[:, k * IC + h * HIC:k * IC + (h + 1) * HIC])
            if k + 1 < NT:
                t2 = k
                for h in range(2):
                    nc.sync.dma_start(out=w_nat[:, t2, h * 640:(h + 1) * 640],
                                      in_=w_r[:, t2, h * 640:(h + 1) * 640])
        for h in range(2):
            nc.sync.dma_start(out=w_nat[:, NT - 1, h * 640:(h + 1) * 640],
                              in_=w_r[:, NT - 1, h * 640:(h + 1) * 640])
        make_identity(nc, ident)
        nc.vector.memset(ones, 1.0)

        # bf16 cast of x for the G-pass (split ACT/DVE, overlaps the load)
        for k in range(K):
            sl = slice(k * IC, (k + 1) * IC)
            if k % 2 == 0:
                nc.scalar.copy(x2b[:, sl], x2[:, sl])
            else:
                nc.vector.tensor_copy(x2b[:, sl], x2[:, sl])

        # ---- build x1 = per-chunk transpose of x2 (PE transpose) ----
        # evacuation alternates ACT/DVE so neither engine paces the PE
        for t in range(NKT):
            ps = pall[:, t % K, :BL]
            nc.tensor.transpose(ps, x2[:, t * 128:(t + 1) * 128], ident)
            if t % 2 == 0:
                nc.scalar.copy(x1[:, t, :], ps)
            else:
                nc.vector.tensor_copy(x1[:, t, :], ps)

        bfulls = {}
        for it in range(NUM_ROUTING):
            if it > 0:
                # ---- softmax over i (given b_full from the AllReduce) ----
                bf_a, bf_b = bfulls[it - 1]
                expb = small.tile([128, NT, J], f32, name=f"expb{it}")
                # exp(b/B): fold the batch-mean 1/B into the exp scale;
                # split so the first half runs while AllReduce B drains
                nc.scalar.activation(
                    expb[:, :NT_A, :].rearrange("p t j -> p (t j)"),
                    bf_a.rearrange("p t j -> p (t j)"),
                    Act.Exp, scale=1.0 / B)
                nc.scalar.activation(
                    expb[:, NT_A:, :].rearrange("p t j -> p (t j)"),
                    bf_b.rearrange("p t j -> p (t j)"),
                    Act.Exp, scale=1.0 / B)
                # Z[j] = sum_i exp(b[i,j]), broadcast to 128 partitions via
                # an accumulating ones-matmul; bank 7 of PSUM
                zp = pall[:, K - 1, :J]
                for t2 in range(NT):
                    nc.tensor.matmul(zp, ones, expb[:, t2, :],
                                     start=(t2 == 0), stop=(t2 == NT - 1))
                zinv = small.tile([128, J], f32, name=f"zinv{it}")
                nc.vector.reciprocal(zinv, zp)
                # crep[i, (j,u)] = expb[i,j] * zinv[j]  (broadcast over u)
                for t2 in range(NT):
                    nc.vector.tensor_mul(
                        crep[:, t2, :].rearrange("p (j u) -> p j u", j=J),
                        expb[:, t2, :].unsqueeze(-1).broadcast_to([128, J, U]),
                        zinv.unsqueeze(-1).broadcast_to([128, J, U]))

            # ---- wp = crep * W on DVE (iters>0). Iteration 0 has uniform
            # c = 1/IC folded into the squash scales, so wp is just a
            # contiguous repack of the strided W view (split ACT/DVE). ----
            for t in range(NKT):
                k, t2 = divmod(t, NT)
                wp_v = wp[:, t, :].rearrange("p (j u) -> p j u", j=J)
                if it == 0:
                    # DVE only: ACT is already saturated in the load phase
                    # with the x2b casts and x1 evacuations
                    nc.vector.tensor_copy(wp_v, w5[:, t2, :, :, k])
                else:
                    nc.vector.tensor_mul(
                        wp_v, w5[:, t2, :, :, k],
                        crep[:, t2, :].rearrange("p (j u) -> p j u", j=J))

            # ---- s = X1^T @ wp : accumulate 72 chunks into PSUM bank 0 ----
            sp = pall[:BL, 0, :JU]
            for t in range(NKT):
                nc.tensor.matmul(sp, x1[:, t, :], wp[:, t, :],
                                 start=(t == 0), stop=(t == NKT - 1))

            # ---- squash (reference quirk: norm over the j axis per (b,u)) ----
            # ACT only does Sqrt here; everything else on DVE to avoid the
            # ~1.3us ACT LUT-table reload per function switch
            s_sb = small.tile([BL, JU], f32, name=f"s_sb{it}")
            nc.vector.tensor_copy(s_sb, sp)
            ssq = small.tile([BL, JU], f32, name=f"ssq{it}")
            nc.vector.tensor_mul(ssq, s_sb, s_sb)
            msq = small.tile([BL, U], f32, name=f"msq{it}")
            nc.vector.tensor_reduce(
                msq, ssq.rearrange("b (j u) -> b u j", j=J),
                axis=mybir.AxisListType.X, op=Alu.add)
            # iteration 0: s here is actually IC*s, so scale m by 1/IC^2 and
            # s by 1/IC while forming v
            sc2 = 1.0 / (IC * IC) if it == 0 else 1.0
            sc1 = 1.0 / IC if it == 0 else 1.0
            mag = small.tile([BL, U], f32, name=f"mag{it}")
            tpb = small.tile([BL, U], f32, name=f"tpb{it}")
            rin = small.tile([BL, U], f32, name=f"rin{it}")
            fv = small.tile([BL, U], f32, name=f"fv{it}")
            nc.scalar.activation(mag, msq, Act.Sqrt, scale=sc2)
            nc.vector.tensor_scalar(tpb, msq, sc2, BETA,
                                    op0=Alu.mult, op1=Alu.add)
            nc.vector.reciprocal(rin, tpb)
            nc.vector.tensor_mul(fv, mag, rin)
            v = small.tile([BL, JU], f32, name=f"v{it}")
            nc.vector.scalar_tensor_tensor(
                out=v.rearrange("b (j u) -> b j u", j=J),
                in0=s_sb.rearrange("b (j u) -> b j u", j=J),
                scalar=sc1,
                in1=fv.unsqueeze(1).broadcast_to([BL, J, U]),
                op0=Alu.mult, op1=Alu.mult)

            if it == NUM_ROUTING - 1:
                nc.sync.dma_start(out=out_flat, in_=v)
                continue
            vb = small.tile([BL, JU], bf16, name=f"vb{it}")
            nc.vector.tensor_copy(vb, v)

            # ---- G = X2^T-chunks @ v, per (t2): 8 banks; ACT evacuates in
            # bank-halves so the next round's matmuls overlap the DVE reads.
            # The b AllReduce is split: half A (t2 < NT_A) is sent as soon as
            # its STT accumulations finish, overlapping the rest of (d). ----
            b_part = small.tile([128, NT, J], f32, name=f"bpart{it}")
            cc_in_a = dram.tile([IC_A, J], f32, name=f"ccina{it}")
            cc_out_a = dram.tile([IC_A, J], f32, name=f"ccouta{it}",
                                 addr_space="Shared")
            cc_in_b = dram.tile([IC - IC_A, J], f32, name=f"ccinb{it}")
            cc_out_b = dram.tile([IC - IC_A, J], f32, name=f"ccoutb{it}",
                                 addr_space="Shared")
            for t2 in range(NT):
                g_sb = scratch.tile([128, K, JU], f32, name="g_sb", bufs=3)
                for h in range(2):
                    for k in range(h * 4, h * 4 + 4):
                        nc.tensor.matmul(
                            pall[:, k, :JU],
                            x2b[:, (k * NT + t2) * 128:
                                (k * NT + t2) * 128 + 128],
                            vb, start=True, stop=True)
                    nc.scalar.copy(g_sb[:, h * 4:h * 4 + 4, :],
                                   pall[:, h * 4:h * 4 + 4, :JU])
                g_fk = g_sb.rearrange("p k f -> p f k")
                for j in range(J):
                    so = scratch.tile([128, U, K], f32, name="stt_scratch")
                    nc.vector.scalar_tensor_tensor(
                        out=so,
                        in0=w5[:, t2, j, :, :],
                        scalar=1.0,
                        in1=g_fk[:, j * U:(j + 1) * U, :],
                        op0=Alu.mult, op1=Alu.mult,
                        accum_out=b_part[:, t2, j:j + 1])
                if t2 == NT_A - 1:
                    nc.sync.dma_start(
                        out=cc_in_a.rearrange("(t p) j -> p t j", p=128),
                        in_=b_part[:, :NT_A, :])
                    nc.gpsimd.collective_compute(
                        "AllReduce", Alu.add,
                        replica_groups=[list(range(NCORES))],
                        ins=[cc_in_a[:, :]], outs=[cc_out_a[:, :]])
            nc.sync.dma_start(
                out=cc_in_b.rearrange("(t p) j -> p t j", p=128),
                in_=b_part[:, NT_A:, :])
            nc.gpsimd.collective_compute(
                "AllReduce", Alu.add,
                replica_groups=[list(range(NCORES))],
                ins=[cc_in_b[:, :]], outs=[cc_out_b[:, :]])
            bf_a = small.tile([128, NT_A, J], f32, name=f"bfa{it}")
            bf_b = small.tile([128, NT - NT_A, J], f32, name=f"bfb{it}")
            nc.sync.dma_start(
                out=bf_a, in_=cc_out_a.rearrange("(t p) j -> p t j", p=128))
            nc.sync.dma_start(
                out=bf_b, in_=cc_out_b.rearrange("(t p) j -> p t j", p=128))
            bfulls[it] = (bf_a, bf_b)

    nc.compile()
    return nc


def _get_nc():
    if "nc" not in _CACHE:
        _CACHE["nc"] = _build_nc()
    return _CACHE["nc"]


def _run(x, W, trace=False, **kw):
    from concourse import bass_utils

    nc = _get_nc()
    x = np.ascontiguousarray(np.asarray(x, dtype=np.float32))
    W = np.ascontiguousarray(np.asarray(W, dtype=np.float32))
    in_maps = [
        {"xs": x[c * BL:(c + 1) * BL], "w": W}
        for c in range(NCORES)
    ]
    res = bass_utils.run_bass_kernel_spmd(
        nc, in_maps, core_ids=list(range(NCORES)), trace=trace, **kw)
    outs = [res.results[c]["out"] for c in range(NCORES)]
    full = np.concatenate(outs, axis=0).reshape(B, J, 4, U // 4)
    return full, res


def kernel(x, W):
    full, _ = _run(x, W, trace=False)
    return full



# revision 6
# speedup vs baseline: 1.4091x; 1.4091x over previous
"""DigitCaps dynamic-routing kernel for Trainium2 (8 NeuronCores, Bass/Tile).

Math (per routing iteration, reformulated to avoid materializing u_hat):
    u_hat[b,i,j,u] = sum_k W[i,j,u,k] * x[b,k,i]
    s[b,ju]  = sum_{ki} X[ki,b] * (c[i,j] * W[ki,ju])          (PE matmul, K=9216)
    v        = squash(s)  with the reference's quirky j-axis norm
    G[ki,ju] = sum_b X[b,ki] * v[b,ju]                         (PE matmul, K=64)
    b[i,j]   = sum_{k,u} W[ki,ju] * G[ki,ju]                   (DVE product+reduce)
    b is AllReduced (sum) over the 8 cores each iteration (batch mean).

Sharding: data-parallel over batch B=512 -> 64 rows per core; W replicated.

v2 design:
  - everything on the PE runs bf16 (fp32 LOW_HIGH matmuls were ~5x slower);
    validated end-to-end L2 err ~3e-3 vs the 2e-2 gate.
  - host pre-casts x/W to bf16 and pre-builds both layouts (natural + transposed
    x, (j,u,k) + (k,j,u) W) so the chip does zero load-phase transposes/casts;
    the load phase is pure DMA (~3.8MB total).
  - softmax uses unnormalized exp: wp = exp(b/B) * W starts per-t2-chunk as
    soon as the AllReduce lands; the 1/Z normalizer is folded into a single
    post-matmul multiply (Z from an accumulating ones-matmul).
  - squash sqrt is a DVE Newton rsqrt (quake seed + 2 iterations), so the ACT
    exp table never reloads (table switches cost ~1.5us each).
  - b-update: G PSUM banks are read directly by a DVE/GPSIMD product into a
    (j,u,k)-ordered bf16 tensor, then one DVE X-reduce per t2 -> b_part.
  - ONE AllReduce per iteration (bf16 payload), plus a tiny warm-up AllReduce
    at t=0 to absorb the ~26us cold-start of the collective stack.
  - dummy warm-up matmuls during the load phase keep the PE HAM un-throttled.
"""

import sys

sys.path.insert(0, "/opt/trn_rl_repo")

from contextlib import ExitStack

import numpy as np

B = 512
NCORES = 8
BL = B // NCORES  # 64 local batch rows
K = 8             # in_units (primary capsule dim)
IC = 1152         # in_channels (number of primary capsules)
J = 10            # num_units (output capsules)
U = 16            # unit_size
JU = J * U        # 160
NT = IC // 128    # 9 i-chunks of 128
NKT = K * NT      # 72 ki-chunks of 128
BETA = 1.45
NUM_ROUTING = 3

_CACHE = {}


def _build_nc():
    import concourse.bass as bass
    import concourse.tile as tile
    from concourse import bacc, mybir

    f32 = mybir.dt.float32
    bf16 = mybir.dt.bfloat16
    i32 = mybir.dt.int32
    Alu = mybir.AluOpType
    Act = mybir.ActivationFunctionType
    Ax = mybir.AxisListType

    nc = bacc.Bacc("TRN2", target_bir_lowering=False, debug=False,
                   num_devices=NCORES)

    # host-prepped bf16 inputs (both layouts, see _prep below)
    xs2 = nc.dram_tensor("xs2", [BL, K * IC], bf16, kind="ExternalInput").ap()
    xs1 = nc.dram_tensor("xs1", [128, NKT, BL], bf16, kind="ExternalInput").ap()
    wn = nc.dram_tensor("wn", [128, NT, JU * K], bf16, kind="ExternalInput").ap()
    wk = nc.dram_tensor("wk", [128, NT, JU * K], bf16, kind="ExternalInput").ap()
    out = nc.dram_tensor("out", [BL, JU], f32, kind="ExternalOutput").ap()

    with tile.TileContext(nc) as tc, ExitStack() as ctx:
        consts = ctx.enter_context(tc.tile_pool(name="consts", bufs=1))
        small = ctx.enter_context(tc.tile_pool(name="small", bufs=2))
        scratch = ctx.enter_context(tc.tile_pool(name="scratch", bufs=8))
        psum = ctx.enter_context(tc.tile_pool(name="psum", bufs=1, space="PSUM"))
        dram = ctx.enter_context(tc.tile_pool(name="dram", bufs=1, space="DRAM"))

        # ---- persistent SBUF tensors ----
        x2b = consts.tile([BL, K * IC], bf16)        # x[b, (k i)] (G stationary)
        x1b = consts.tile([128, NKT, BL], bf16)      # x^T per ki-chunk (s stationary)
        w_natb = consts.tile([128, NT, JU * K], bf16)  # W[(i),(j,u,k)]
        w_kju = consts.tile([128, NT, K * JU], bf16)   # W[(i),(k,j,u)]
        wp = consts.tile([128, NT, K * JU], bf16)      # exp-scaled W (iters>0)
        ones = consts.tile([128, 128], bf16)         # Z broadcast matmul lhsT
        wm = consts.tile([128, 512], bf16)           # PE warm-up garbage

        # one PSUM tensor = all 8 banks; everything slices into it
        pall = psum.tile([128, K, 512], f32)

        nc.vector.memset(ones, 1.0)
        nc.gpsimd.memset(wm, 0.001)

        # ---- PE warm-up: keep HAM at K=8/8 through the load phase ----
        for i in range(30):
            nc.tensor.matmul(pall[:, 7, :512], wm[:, :128], wm,
                             start=True, stop=True)

        # ---- ACT exp-table preload (the only ACT table this kernel uses) ----
        etp = consts.tile([BL, 1], f32)
        nc.vector.memset(etp, 0.0)
        nc.scalar.activation(etp, etp, Act.Exp, scale=1.0)

        # ---- collective warm-up: absorb the ncfw cold-start latency ----
        warm_in = dram.tile([128, 1], f32, name="warm_in")
        warm_out = dram.tile([128, 1], f32, name="warm_out",
                             addr_space="Shared")
        wz = consts.tile([128, 1], f32)
        nc.vector.memset(wz, 0.0)
        nc.sync.dma_start(out=warm_in, in_=wz)
        nc.gpsimd.collective_compute(
            "AllReduce", Alu.add,
            replica_groups=[list(range(NCORES))],
            ins=[warm_in[:, :]], outs=[warm_out[:, :]])

        # ---- loads: split into pieces so transfers spread across the
        # 16 DMA engines (a single transfer lands on one engine @~22GB/s) ----
        for k in range(K):
            nc.sync.dma_start(out=x2b[:, k * IC:(k + 1) * IC],
                              in_=xs2[:, k * IC:(k + 1) * IC])
            nc.sync.dma_start(out=x1b[:, k * NT:(k + 1) * NT, :],
                              in_=xs1[:, k * NT:(k + 1) * NT, :])
        for t2 in range(NT):
            nc.sync.dma_start(out=w_kju[:, t2, :], in_=wk[:, t2, :])
            nc.sync.dma_start(out=w_natb[:, t2, :], in_=wn[:, t2, :])

        # 4D views for the b-update product
        w4 = w_natb.rearrange("p t (j u k) -> p t j u k", j=J, u=U)

        bf_tiles = {}
        for it in range(NUM_ROUTING):
            # ---- wp = exp(b_sum/B) * w_kju (iters>0). Iteration 0 has
            # uniform c = 1/IC folded into the squash scales, so the matmul
            # rhs is just w_kju directly. ----
            if it > 0:
                bf_t = bf_tiles[it - 1]
                expb = small.tile([128, NT, J], bf16, name=f"expb{it}")
                for t2 in range(NT):
                    nc.scalar.activation(expb[:, t2, :], bf_t[:, t2, :],
                                         Act.Exp, scale=1.0 / B)
                    nc.vector.tensor_mul(
                        wp[:, t2, :].rearrange("p (k j u) -> p k j u",
                                               k=K, j=J),
                        w_kju[:, t2, :].rearrange("p (k j u) -> p k j u",
                                                  k=K, j=J),
                        expb[:, t2, :].unsqueeze(1).unsqueeze(-1)
                        .broadcast_to([128, K, J, U]))
                rhs_src = wp
            else:
                rhs_src = w_kju

            # ---- s = X1^T @ wp : accumulate 72 chunks into PSUM bank 0 ----
            sp = pall[:BL, 0, :JU]
            for t2 in range(NT):
                for k in range(K):
                    t = k * NT + t2
                    first = (t2 == 0 and k == 0)
                    last = (t2 == NT - 1 and k == K - 1)
                    nc.tensor.matmul(sp, x1b[:, t, :],
                                     rhs_src[:, t2, k * JU:(k + 1) * JU],
                                     start=first, stop=last)

            # ---- Z[j] = sum_i exp(b[i,j]) via accumulating ones-matmul ----
            if it > 0:
                zp = pall[:, 1, :J]
                for t2 in range(NT):
                    nc.tensor.matmul(zp, ones, expb[:, t2, :],
                                     start=(t2 == 0), stop=(t2 == NT - 1))
                zinv = small.tile([BL, J], f32, name=f"zinv{it}")
                nc.vector.reciprocal(zinv, zp[:BL, :])
                # s_norm = s * (1/Z_j), also evacuates PSUM
                s_sb = small.tile([BL, JU], f32, name=f"s_sb{it}")
                nc.vector.tensor_mul(
                    s_sb.rearrange("b (j u) -> b j u", j=J),
                    sp.rearrange("b (j u) -> b j u", j=J),
                    zinv.unsqueeze(-1).broadcast_to([BL, J, U]))
            else:
                s_sb = small.tile([BL, JU], f32, name=f"s_sb{it}")
                nc.vector.tensor_copy(s_sb, sp)

            # ---- squash (reference quirk: norm over the j axis per (b,u)).
            # All on DVE; sqrt via quake-seed Newton rsqrt (no ACT tables). ----
            ssq = small.tile([BL, JU], f32, name=f"ssq{it}")
            nc.vector.tensor_mul(ssq, s_sb, s_sb)
            msq = small.tile([BL, U], f32, name=f"msq{it}")
            nc.vector.tensor_reduce(
                msq, ssq.rearrange("b (j u) -> b u j", j=J),
                axis=Ax.X, op=Alu.add)
            # iteration 0: s here is actually IC*s, so scale msq by 1/IC^2
            # and fold 1/IC into the final v multiply
            sc2 = 1.0 / (IC * IC) if it == 0 else 1.0
            scv = 1.0 / (IC * IC) if it == 0 else 1.0
            # y ~= rsqrt(msq): quake seed + 2 Newton iterations
            ti = small.tile([BL, U], i32, name=f"ti{it}")
            nc.vector.tensor_scalar(ti, msq.bitcast(i32), 1, 0,
                                    op0=Alu.arith_shift_right,
                                    op1=Alu.logical_shift_left)
            y0i = small.tile([BL, U], i32, name=f"y0i{it}")
            nc.vector.tensor_scalar(y0i, ti, 0x5f3759df, -1,
                                    op0=Alu.subtract, op1=Alu.mult)
            y0 = y0i.bitcast(f32)
            half = small.tile([BL, U], f32, name=f"half{it}")
            nc.vector.tensor_scalar(half, msq, 0.5, 0.0,
                                    op0=Alu.mult, op1=Alu.add)
            yc = y0
            for n in range(2):
                t_a = small.tile([BL, U], f32, name=f"na{it}_{n}")
                nc.vector.tensor_mul(t_a, yc, yc)
                nc.vector.tensor_mul(t_a, t_a, half)
                nc.vector.tensor_scalar(t_a, t_a, -1.0, 1.5,
                                        op0=Alu.mult, op1=Alu.add)
                t_b = small.tile([BL, U], f32, name=f"nb{it}_{n}")
                nc.vector.tensor_mul(t_b, yc, t_a)
                yc = t_b
            # f = msq*y * 1/(beta + msq*sc2) (scaled for it0)
            tpb = small.tile([BL, U], f32, name=f"tpb{it}")
            nc.vector.tensor_scalar(tpb, msq, sc2, BETA,
                                    op0=Alu.mult, op1=Alu.add)
            rin = small.tile([BL, U], f32, name=f"rin{it}")
            nc.vector.reciprocal(rin, tpb)
            fv = small.tile([BL, U], f32, name=f"fv{it}")
            nc.vector.tensor_mul(fv, msq, yc)
            nc.vector.tensor_mul(fv, fv, rin)
            v = small.tile([BL, JU], f32, name=f"v{it}")
            nc.vector.scalar_tensor_tensor(
                out=v.rearrange("b (j u) -> b j u", j=J),
                in0=s_sb.rearrange("b (j u) -> b j u", j=J),
                scalar=scv,
                in1=fv.unsqueeze(1).broadcast_to([BL, J, U]),
                op0=Alu.mult, op1=Alu.mult)

            if it == NUM_ROUTING - 1:
                nc.sync.dma_start(out=out, in_=v)
                continue
            vb = small.tile([BL, JU], bf16, name=f"vb{it}")
            nc.vector.tensor_copy(vb, v)

            # ---- G = X2^T-chunks @ v per t2; banks ping-pong in halves
            # (even t2 -> banks 0-3, odd -> 4-7; k packed 2-per-bank).
            # b_part[:, t2, j] = sum_{u,k} W * G via product + X-reduce. ----
            b_part = small.tile([128, NT, J], f32, name=f"bpart{it}")
            b_bf = small.tile([128, NT, J], bf16, name=f"bbf{it}")
            for t2 in range(NT):
                b0 = 0 if t2 % 2 == 0 else 4
                for k in range(K):
                    bank = b0 + k // 2
                    kk = k % 2
                    nc.tensor.matmul(
                        pall[:, bank, kk * JU:(kk + 1) * JU],
                        x2b[:, (k * IC + t2 * 128):(k * IC + t2 * 128) + 128],
                        vb, start=True, stop=True)
                # product P[(j,u,k)] = W * G, G read straight out of PSUM
                # via a 4D AP [j, u, bank, kk]
                pg = pall[:, b0:b0 + 4, :2 * JU].rearrange(
                    "p b (kk j u) -> p j u b kk", kk=2, j=J)
                prod = scratch.tile([128, JU * K], bf16, name="prod", bufs=3)
                p4 = prod.rearrange("p (j u b kk) -> p j u b kk", j=J, u=U, b=4)
                # DVE multiplies j0-4 straight out of PSUM; ACT evacuates
                # j5-9 to SBUF for GPSIMD (GPSIMD cannot read PSUM)
                JH = J // 2
                g5h = scratch.tile([128, JH * U * K], bf16, name="g5h", bufs=3)
                g5h4 = g5h.rearrange("p (j u b kk) -> p j u b kk",
                                     j=JH, u=U, b=4)
                nc.scalar.copy(g5h4, pg[:, JH:])
                nc.vector.tensor_mul(p4[:, :JH], w4[:, t2, :JH]
                                     .rearrange("p j u (b kk) -> p j u b kk",
                                                b=4),
                                     pg[:, :JH])
                nc.gpsimd.tensor_mul(p4[:, JH:], w4[:, t2, JH:]
                                     .rearrange("p j u (b kk) -> p j u b kk",
                                                b=4),
                                     g5h4)
                nc.vector.tensor_reduce(
                    b_part[:, t2, :],
                    prod.rearrange("p (j x) -> p j x", j=J),
                    axis=Ax.X, op=Alu.add)
                nc.scalar.copy(b_bf[:, t2, :], b_part[:, t2, :])

            # ---- AllReduce b over the 8 cores (bf16 payload) ----
            cc_in = dram.tile([IC, J], bf16, name=f"ccin{it}")
            cc_out = dram.tile([IC, J], bf16, name=f"ccout{it}",
                               addr_space="Shared")
            nc.sync.dma_start(
                out=cc_in.rearrange("(t p) j -> p t j", p=128),
                in_=b_bf)
            nc.gpsimd.collective_compute(
                "AllReduce", Alu.add,
                replica_groups=[list(range(NCORES))],
                ins=[cc_in[:, :]], outs=[cc_out[:, :]])
            bf_t = small.tile([128, NT, J], bf16, name=f"bf{it}")
            nc.sync.dma_start(
                out=bf_t, in_=cc_out.rearrange("(t p) j -> p t j", p=128))
            bf_tiles[it] = bf_t

    nc.compile()
    return nc


def _prep(x, W):
    """Host-side prep: bf16 cast + both layouts for x and W."""
    import ml_dtypes

    bf16 = ml_dtypes.bfloat16
    x = np.asarray(x, dtype=np.float32)
    W = np.asarray(W, dtype=np.float32)
    xb = x.astype(bf16)                      # (B, K, IC)
    # natural: [b, (k i)]
    xs2 = np.ascontiguousarray(xb.reshape(B, K * IC))
    # transposed: [p, (k t2), b] per core handled by slicing b below
    # W natural (j,u,k): [p, t2, (j u k)]
    wn = np.ascontiguousarray(
        W.reshape(NT, 128, J * U * K).transpose(1, 0, 2).astype(bf16))
    # W (k,j,u): [p, t2, (k j u)]
    wk = np.ascontiguousarray(
        W.reshape(NT, 128, J, U, K).transpose(1, 0, 4, 2, 3)
        .reshape(128, NT, K * J * U).astype(bf16))
    in_maps = []
    for c in range(NCORES):
        rows = xb[c * BL:(c + 1) * BL]       # (BL, K, IC)
        xs1 = np.ascontiguousarray(
            rows.reshape(BL, K, NT, 128).transpose(3, 1, 2, 0)
            .reshape(128, NKT, BL))
        in_maps.append({
            "xs2": xs2[c * BL:(c + 1) * BL],
            "xs1": xs1,
            "wn": wn,
            "wk": wk,
        })
    return in_maps


def _run(x, W, trace=False, **kw):
    from concourse import bass_utils

    nc = _get_nc()
    in_maps = _prep(x, W)
    res = bass_utils.run_bass_kernel_spmd(
        nc, in_maps, core_ids=list(range(NCORES)), trace=trace, **kw)
    outs = [res.results[c]["out"] for c in range(NCORES)]
    full = np.concatenate(outs, axis=0).reshape(B, J, 4, U // 4)
    return full, res


def _get_nc():
    if "nc" not in _CACHE:
        _CACHE["nc"] = _build_nc()
    return _CACHE["nc"]


def kernel(x, W):
    full, _ = _run(x, W, trace=False)
    return full
